# revision 18
# baseline (speedup 1.0000x reference)
"""Multi-head attention (softmax over the QUERY axis) on 8 TRN2 NeuronCores.

Problem shapes: Q [T=1024, B=8, D=256]; per-head full-width projections
Wq/Wk/Wv [H=8, E=512, D=256]; Wo [D=256, H*E=4096].

Sharding: data-parallel over batch B — core b computes all H heads for
batch b. No collectives; the host re-stacks per-core outputs along B.

Math restructuring (all exact algebra, validated to rel_err 0.0037):
  * Scores are a QUADRATIC FORM:  A[t,s] = (Q M Q^T)[t,s] + c1[t] + c2[s] + c0
    with M = Wq^T Wk [256x256] precomputed on the host. On-chip this is
    G~ = Q M + 1*w2^T (folds the c2 term), then AT[s,t] = sum_d Q[s,d] G~[t,d]
    — contraction 256 instead of 512, and the q/k projections disappear.
    c0 folds into the Exp bias; the per-query term becomes a multiplicative
    factor f[t] = exp(scale*c1[t]) (host-computed) applied on the DVE by a
    fused multiply+row-sum (affine_mul_reduce) that also produces l[s].
  * Wv and Wo are fused on the host: u_h = Q @ (Wo_h @ Wv_h)^T + Wo_h@bv_h,
    so the attention-output matmul contracts against [*,256] instead of
    [*,512] and the output projection disappears. bvo enters via a K=1
    matmul into the same PSUM accumulation. The output leaves the chip
    TRANSPOSED ([d, t]); the host untransposes and adds bo.

Per-core per-head engine schedule (PSUM: 4 score banks, 2 proj, 2 attn):
  G~[h+1] production is emitted between head h's score loop and its
  attention matmuls so the PE never waits on the exp drain tail.
"""

import sys

sys.path.insert(0, "/opt/trn_rl_repo")

from contextlib import ExitStack

import ml_dtypes
import numpy as np

import concourse.bass as bass
import concourse.tile as tile
from concourse.tile import add_dep_helper
from concourse import bacc, bass_utils, mybir

T, B, D, H, E = 1024, 8, 256, 8, 512
N_CORES = 8

F32 = mybir.dt.float32
BF16 = mybir.dt.bfloat16
FP8 = mybir.dt.float8e4
AF = mybir.ActivationFunctionType
DR = mybir.MatmulPerfMode.DoubleRow


def _bcast(ap_row, parts):
    """Partition-broadcast a [1, n] DRAM AP to [parts, n] (step-0 partition)."""
    return bass.AP(
        tensor=ap_row.tensor,
        offset=ap_row.offset,
        ap=[[0, parts], list(ap_row.ap[-1])],
    )


def build_nc(t=T, d=D, h=H, e=E):
    """Build the per-core SPMD program. Returns a compiled Bacc."""
    TC = t // 512   # t chunks (512-wide matmul free dim)
    SB = t // 128   # s blocks
    DC = d // 128   # d chunks (contraction)
    DB = d // 128   # d blocks of the transposed output
    scale = float(1.0 / np.sqrt(e))

    nc = bacc.Bacc("TRN2", target_bir_lowering=False, debug=False)

    qt_d = nc.dram_tensor("qt", [128, DC, t], BF16, kind="ExternalInput").ap()
    qt8_d = nc.dram_tensor("qt8", [128, DC, t], FP8, kind="ExternalInput").ap()
    mt_d = nc.dram_tensor("mt", [h, 128, DC, d], BF16, kind="ExternalInput").ap()
    wvot_d = nc.dram_tensor("wvot", [h, 128, DC, d], BF16, kind="ExternalInput").ap()
    w2_d = nc.dram_tensor("w2s", [128, h, DB], F32, kind="ExternalInput").ap()
    bc0_d = nc.dram_tensor("bc0", [128, h], F32, kind="ExternalInput").ap()
    c1_d = nc.dram_tensor("c1r", [1, h, t], BF16, kind="ExternalInput").ap()
    bvo_d = nc.dram_tensor("bvo", [1, h, d], BF16, kind="ExternalInput").ap()
    out_d = nc.dram_tensor("out", [128, DB, t], F32, kind="ExternalOutput").ap()

    with tile.TileContext(nc) as tc, ExitStack() as ctx:
        consts = ctx.enter_context(tc.tile_pool(name="consts", bufs=1))
        wpool = ctx.enter_context(tc.tile_pool(name="wpool", bufs=2))
        hpool = ctx.enter_context(tc.tile_pool(name="hpool", bufs=2))
        spool = ctx.enter_context(tc.tile_pool(name="spool", bufs=2))
        at_pool = ctx.enter_context(tc.tile_pool(name="at_pool", bufs=2, space="PSUM"))
        mm_pool = ctx.enter_context(tc.tile_pool(name="mm_pool", bufs=2, space="PSUM"))
        ao_pool = ctx.enter_context(tc.tile_pool(name="ao_pool", bufs=2, space="PSUM"))

        # ---- persistent loads -------------------------------------------
        qt_sb = consts.tile([128, DC, t], BF16)
        nc.sync.dma_start(out=qt_sb[:, 0, :], in_=qt_d[:, 0, :])
        qt8_sb = consts.tile([128, DC, t], FP8)
        w2_sb = consts.tile([128, h, DB], F32)
        nc.sync.dma_start(out=w2_sb, in_=w2_d)
        bc0_sb = consts.tile([128, h], F32)
        nc.sync.dma_start(out=bc0_sb, in_=bc0_d)
        ones_sb = consts.tile([1, 128], BF16)
        nc.vector.memset(ones_sb, 1.0)
        bvo_sb = consts.tile([1, h, d], BF16)
        nc.sync.dma_start(out=bvo_sb, in_=bvo_d)
        c1_sb = consts.tile([1, h, t], BF16)
        nc.sync.dma_start(out=c1_sb, in_=c1_d)
        out_sb = consts.tile([128, DB, t], F32)

        # ---- PE warm-up during the initial DMA wait ---------------------
        scratch = consts.tile([128, 640], BF16)
        nc.vector.memset(scratch, 0.0)
        warm_g = consts.tile([128, 512], BF16)
        ps_w = mm_pool.tile([128, 512], F32, tag="mm")
        for _ in range(14):
            nc.tensor.matmul(
                ps_w, scratch[:, :128], scratch[:, 128:640], start=True, stop=True
            )
        # preload the Exp spline table off the critical path
        nc.scalar.activation(warm_g, ps_w, AF.Exp)

        def make_gt(hh, mt_sb):
            """G~^T[do, t] = sum_di M[di,do] Q^T[di,t] + w2[do], bf16."""
            gt = hpool.tile([128, DC, t], FP8, name=f"gt{hh}")
            first = None
            for dob in range(DC):
                for tch in range(TC):
                    tsl = slice(tch * 512, (tch + 1) * 512)
                    ps_g = mm_pool.tile([128, 512], F32, tag="mm")
                    for dci in range(DC):
                        mm = nc.tensor.matmul(
                            ps_g,
                            mt_sb[:, dci, dob * 128 : (dob + 1) * 128],
                            qt_sb[:, dci, tsl],
                            start=(dci == 0),
                            stop=(dci == DC - 1),
                        )
                        if first is None:
                            first = mm
                    if dob == 0 and tch == 0:
                        nc.scalar.activation(
                            gt[:, dob, tsl],
                            ps_g,
                            AF.Identity,
                            bias=w2_sb[:, hh, dob : dob + 1],
                        )
                    else:
                        nc.vector.tensor_scalar_add(
                            gt[:, dob, tsl], ps_g, w2_sb[:, hh, dob : dob + 1]
                        )
            return gt, first

        mt_cur = wpool.tile([128, DC, d], BF16, name="mt0")
        nc.sync.dma_start(out=mt_cur, in_=mt_d[0])
        nc.sync.dma_start(out=qt_sb[:, 1, :], in_=qt_d[:, 1, :])
        nc.sync.dma_start(out=qt8_sb, in_=qt8_d)
        gt_cur, first_mm = make_gt(0, mt_cur)

        for hh in range(h):
            # ---- per-head bulk loads (prefetched via wpool) -------------
            gated = []
            wvo_sb = wpool.tile([128, DC, d], BF16, name=f"wvo{hh}")
            gated.append(nc.sync.dma_start(out=wvo_sb, in_=wvot_d[hh]))
            if hh == 0:
                for g in gated:
                    add_dep_helper(
                        g.ins, first_mm.ins, reason="defer bulk load past cold start"
                    )
            if hh + 1 < h:
                mt_next = wpool.tile([128, DC, d], BF16, name=f"mt{hh + 1}")
                nc.sync.dma_start(out=mt_next, in_=mt_d[hh + 1])

            # ---- scores + exp + f/rowsum + fused U ----------------------
            Ex = hpool.tile([128, SB, t], BF16)
            Uv = hpool.tile([128, SB, d], BF16)
            lsum = spool.tile([128, SB], F32)
            rr = spool.tile([128, SB], F32)
            for sb in range(SB):
                ssl = slice(sb * 128, (sb + 1) * 128)
                at = at_pool.tile([128, TC * 512], F32, tag="at", name="at")
                # fp8 DoubleRow: contraction 256 in one pass, 256-wide out
                # chunks; start=True only on each bank's first MM (start
                # clears the whole 2KB bank region).
                for q4 in range(TC * 2):
                    nc.tensor.matmul(
                        at[:, q4 * 256 : (q4 + 1) * 256],
                        qt8_sb[:, 0:DC, ssl],
                        gt_cur[:, 0:DC, q4 * 256 : (q4 + 1) * 256],
                        start=(q4 % 2 == 0),
                        stop=False,
                        perf_mode=DR,
                        skip_group_check=True,
                    )
                # c1[t] enters the logits as a K=1 rank-1 accumulate
                for q4 in range(TC * 2):
                    nc.tensor.matmul(
                        at[:, q4 * 256 : (q4 + 1) * 256],
                        ones_sb,
                        c1_sb[:, hh, q4 * 256 : (q4 + 1) * 256],
                        start=False,
                        stop=(q4 % 2 == 1),
                        skip_group_check=True,
                    )
                # fused U projection for this s-block (fills PE bubbles);
                # bvo enters as a K=1 matmul row
                ps_u = mm_pool.tile([128, 512], F32, tag="mm")
                nc.tensor.matmul(
                    ps_u[:, :d],
                    ones_sb,
                    bvo_sb[:, hh, :],
                    start=True,
                    stop=False,
                    skip_group_check=True,
                )
                for dc in range(DC):
                    nc.tensor.matmul(
                        ps_u[:, :d],
                        qt_sb[:, dc, ssl],
                        wvo_sb[:, dc, :],
                        start=False,
                        stop=(dc == DC - 1),
                        skip_group_check=True,
                    )
                nc.scalar.activation(
                    Ex[:, sb, :],
                    at,
                    AF.Exp,
                    scale=scale,
                    bias=bc0_sb[:, hh : hh + 1],
                    accum_out=lsum[:, sb : sb + 1],
                )
                nc.vector.reciprocal(rr[:, sb : sb + 1], lsum[:, sb : sb + 1])
                nc.vector.tensor_scalar_mul(
                    Uv[:, sb, :], ps_u[:, :d], rr[:, sb : sb + 1]
                )

            # ---- next head's G~ while ScalarE drains the exp tail -------
            if hh + 1 < h:
                gt_cur, _ = make_gt(hh + 1, mt_next)
                mt_cur = mt_next

            # ---- attention output: PSUM over s-blocks, SBUF over heads --
            for db in range(DB):
                dsl = slice(db * 128, (db + 1) * 128)
                pss = [
                    ao_pool.tile([128, 512], F32, tag="ao", name=f"ao{i}")
                    for i in range(TC)
                ]
                for sc in range(SB):
                    for tch in range(TC):
                        tsl = slice(tch * 512, (tch + 1) * 512)
                        nc.tensor.matmul(
                            pss[tch],
                            Uv[:, sc, dsl],
                            Ex[:, sc, tsl],
                            start=(sc == 0),
                            stop=(sc == SB - 1),
                        )
                for tch in range(TC):
                    tsl = slice(tch * 512, (tch + 1) * 512)
                    if hh == 0:
                        nc.vector.tensor_copy(out_sb[:, db, tsl], pss[tch])
                    else:
                        nc.vector.tensor_add(
                            out_sb[:, db, tsl], out_sb[:, db, tsl], pss[tch]
                        )
                    if hh == h - 1:
                        nc.sync.dma_start(
                            out=out_d[:, db, tsl], in_=out_sb[:, db, tsl]
                        )

    nc.compile()
    return nc


_NC_CACHE = {}


def _get_nc(shape_key):
    if shape_key not in _NC_CACHE:
        _NC_CACHE[shape_key] = build_nc(*shape_key)
    return _NC_CACHE[shape_key]


def _pmajor(a, last):
    """[..., C*128, last] -> [..., 128, C, last] partition-major layout."""
    lead = a.shape[:-2]
    c = a.shape[-2] // 128
    return np.ascontiguousarray(
        a.reshape(*lead, c, 128, last).swapaxes(-3, -2)
    )


def _prep_inputs(Q, Wq, bq, Wk, bk, Wv, bv, Wo, bo):
    t, b, d = Q.shape
    h, e, _ = Wq.shape
    s = np.float32(1.0 / np.sqrt(e))
    bf = ml_dtypes.bfloat16
    Q = np.asarray(Q, np.float32)
    Wq = np.asarray(Wq, np.float32)
    Wk = np.asarray(Wk, np.float32)
    Wv = np.asarray(Wv, np.float32)
    Wo = np.asarray(Wo, np.float32)
    bq = np.asarray(bq, np.float32)
    bk = np.asarray(bk, np.float32)
    bv = np.asarray(bv, np.float32)

    # quadratic-form fold: scores need M, w2 (into G~), c1 -> f, c0 -> bias
    M = np.stack([Wq[i].T @ Wk[i] for i in range(h)])           # [H, D, D]
    w1 = np.stack([Wq[i].T @ bk[i] for i in range(h)])          # [H, D]
    w2 = np.stack([Wk[i].T @ bq[i] for i in range(h)])          # [H, D]
    c0 = np.array([bq[i] @ bk[i] for i in range(h)], np.float32)
    # fused V/O projection
    Wvo = np.stack([(Wo[:, i * e : (i + 1) * e] @ Wv[i]).T for i in range(h)])
    bvo = np.stack([Wo[:, i * e : (i + 1) * e] @ bv[i] for i in range(h)])

    qt_all = _pmajor(Q.transpose(1, 2, 0).astype(bf), t)        # [B,128,DC,T]
    f8 = ml_dtypes.float8_e4m3
    qt8_all = _pmajor(
        np.clip(Q.transpose(1, 2, 0), -240, 240).astype(f8), t
    )
    # c1[t] = Q @ w1 per batch & head, added to the raw logits on-chip
    c1_all = np.einsum("tbd,hd->bht", Q, w1).astype(bf)  # [B,H,T]
    shared = {
        "mt": _pmajor(M.astype(bf), d),
        "wvot": _pmajor(Wvo.astype(bf), d),
        "w2s": np.ascontiguousarray(w2.reshape(h, -1, 128).transpose(2, 0, 1)),
        "bc0": np.ascontiguousarray(
            np.tile((s * c0)[None, :], (128, 1)).astype(np.float32)
        ),
        "bvo": np.ascontiguousarray(bvo.astype(bf)[None]),
    }
    in_maps = [
        {
            "qt": np.ascontiguousarray(qt_all[bb]),
            "qt8": np.ascontiguousarray(qt8_all[bb]),
            "c1r": np.ascontiguousarray(c1_all[bb][None]),
            **shared,
        }
        for bb in range(b)
    ]
    return in_maps, (t, d, h, e)


def kernel(Q, Wq, bq, Wk, bk, Wv, bv, Wo, bo, _trace=False):
    in_maps, (t, d, h, e) = _prep_inputs(Q, Wq, bq, Wk, bk, Wv, bv, Wo, bo)
    bo_f = np.asarray(bo, np.float32)
    nc = _get_nc((t, d, h, e))
    res = bass_utils.run_bass_kernel_spmd(
        nc, in_maps, core_ids=list(range(len(in_maps))), trace=_trace
    )
    # device output is OUT[d, t] partition-major: [128, DB, t]
    outs = []
    for b in range(len(in_maps)):
        arr = res.results[b]["out"]  # [128, DB, t]
        outs.append(arr.transpose(2, 1, 0).reshape(t, d) + bo_f)
    out = np.stack(outs, axis=1)
    if _trace:
        kernel.last_results = res
    return np.ascontiguousarray(out.astype(np.float32))


# revision 19
# speedup vs baseline: 1.1181x; 1.1181x over previous
"""Multi-head attention (softmax over the QUERY axis) on 8 TRN2 NeuronCores.

Problem shapes: Q [T=1024, B=8, D=256]; per-head full-width projections
Wq/Wk/Wv [H=8, E=512, D=256]; Wo [D=256, H*E=4096].

Sharding: data-parallel over batch B — core b computes all H heads for
batch b. No collectives; the host re-stacks per-core outputs along B.

Math restructuring (exact algebra, validated numerically):
  * Scores are a QUADRATIC FORM:  A[t,s] = (Q M Q^T)[t,s] + c1[t] + c2[s] + c0
    with M = Wq^T Wk [256x256] precomputed on the host. On-chip this is
    G~ = Q M + 1*w2^T (folds the c2 term), then AT[s,t] = sum_d Q[s,d] G~[t,d]
    — contraction 256 instead of 512, and the q/k projections disappear.
    The AT matmul runs in fp8e4m3 DoubleRow (256-wide output chunks).
    c0 folds into the Exp bias; the per-query term becomes a multiplicative
    factor f[t] = exp(scale*c1[t]) (host-computed) applied on the DVE by a
    fused multiply+row-sum (scalar_tensor_tensor) that also produces l[s].
  * Wv and Wo are fused on the host: u_h = Q @ (Wo_h @ Wv_h)^T + Wo_h@bv_h,
    so the attention-output matmul contracts against [*,256] instead of
    [*,512] and the output projection disappears. bvo enters via a K=1
    matmul into the same PSUM accumulation. The output leaves the chip
    TRANSPOSED ([d, t]); the host untransposes and adds bo.

Scheduling: the attention matmuls for the first output d-block chase the
score pipeline inside the s-block loop (lag 2), so the PE fills the
ScalarE/DVE-gated scores window; the second d-block and the next head's
G~ production form the tail.
"""

import sys

sys.path.insert(0, "/opt/trn_rl_repo")

from contextlib import ExitStack

import ml_dtypes
import numpy as np

import concourse.bass as bass
import concourse.tile as tile
from concourse.tile import add_dep_helper
from concourse import bacc, bass_utils, mybir

T, B, D, H, E = 1024, 8, 256, 8, 512
N_CORES = 8

F32 = mybir.dt.float32
BF16 = mybir.dt.bfloat16
FP8 = mybir.dt.float8e4
AF = mybir.ActivationFunctionType
DR = mybir.MatmulPerfMode.DoubleRow


def _bcast(ap_row, parts):
    """Partition-broadcast a [1, n] DRAM AP to [parts, n] (step-0 partition)."""
    return bass.AP(
        tensor=ap_row.tensor,
        offset=ap_row.offset,
        ap=[[0, parts], list(ap_row.ap[-1])],
    )


def build_nc(t=T, d=D, h=H, e=E):
    """Build the per-core SPMD program. Returns a compiled Bacc."""
    TC = t // 512   # t chunks (512-wide matmul free dim)
    SB = t // 128   # s blocks
    DC = d // 128   # d chunks (contraction)
    DB = d // 128   # d blocks of the transposed output
    scale = float(1.0 / np.sqrt(e))

    nc = bacc.Bacc("TRN2", target_bir_lowering=False, debug=False)

    qt_d = nc.dram_tensor("qt", [128, DC, t], BF16, kind="ExternalInput").ap()
    qt8_d = nc.dram_tensor("qt8", [128, DC, t], FP8, kind="ExternalInput").ap()
    mt_d = nc.dram_tensor("mt", [h, 128, DC, d], BF16, kind="ExternalInput").ap()
    wvot_d = nc.dram_tensor("wvot", [h, 128, DC, d], BF16, kind="ExternalInput").ap()
    w2_d = nc.dram_tensor("w2s", [128, h, DB], F32, kind="ExternalInput").ap()
    bc0_d = nc.dram_tensor("bc0", [128, h], F32, kind="ExternalInput").ap()
    f_d = nc.dram_tensor("fq", [h, t], BF16, kind="ExternalInput").ap()
    bvo_d = nc.dram_tensor("bvo", [1, h, d], BF16, kind="ExternalInput").ap()
    out_d = nc.dram_tensor("out", [128, DB, t], F32, kind="ExternalOutput").ap()

    with tile.TileContext(nc) as tc, ExitStack() as ctx:
        consts = ctx.enter_context(tc.tile_pool(name="consts", bufs=1))
        wpool = ctx.enter_context(tc.tile_pool(name="wpool", bufs=2))
        hpool = ctx.enter_context(tc.tile_pool(name="hpool", bufs=2))
        spool = ctx.enter_context(tc.tile_pool(name="spool", bufs=2))
        at_pool = ctx.enter_context(tc.tile_pool(name="at_pool", bufs=2, space="PSUM"))
        mm_pool = ctx.enter_context(tc.tile_pool(name="mm_pool", bufs=2, space="PSUM"))
        ao_pool = ctx.enter_context(tc.tile_pool(name="ao_pool", bufs=2, space="PSUM"))

        # ---- persistent loads -------------------------------------------
        qt_sb = consts.tile([128, DC, t], BF16)
        nc.sync.dma_start(out=qt_sb[:, 0, :], in_=qt_d[:, 0, :])
        qt8_sb = consts.tile([128, DC, t], FP8)
        w2_sb = consts.tile([128, h, DB], F32)
        nc.sync.dma_start(out=w2_sb, in_=w2_d)
        bc0_sb = consts.tile([128, h], F32)
        nc.sync.dma_start(out=bc0_sb, in_=bc0_d)
        ones_sb = consts.tile([1, 128], BF16)
        nc.vector.memset(ones_sb, 1.0)
        bvo_sb = consts.tile([1, h, d], BF16)
        nc.sync.dma_start(out=bvo_sb, in_=bvo_d)
        out_sb = consts.tile([128, DB, t], F32)

        # ---- PE warm-up during the initial DMA wait ---------------------
        scratch = consts.tile([128, 640], BF16)
        nc.vector.memset(scratch, 0.0)
        warm_g = consts.tile([128, 512], BF16)
        ps_w = mm_pool.tile([128, 512], F32, tag="mm")
        for _ in range(14):
            nc.tensor.matmul(
                ps_w, scratch[:, :128], scratch[:, 128:640], start=True, stop=True
            )
        # preload the Exp spline table off the critical path
        nc.scalar.activation(warm_g, ps_w, AF.Exp)

        def make_gt(hh, mt_sb):
            """G~^T[do, t] = sum_di M[di,do] Q^T[di,t] + w2[do], fp8."""
            gt = hpool.tile([128, DC, t], FP8, name=f"gt{hh}")
            first = None
            for dob in range(DC):
                for tch in range(TC):
                    tsl = slice(tch * 512, (tch + 1) * 512)
                    ps_g = mm_pool.tile([128, 512], F32, tag="mm")
                    for dci in range(DC):
                        mm = nc.tensor.matmul(
                            ps_g,
                            mt_sb[:, dci, dob * 128 : (dob + 1) * 128],
                            qt_sb[:, dci, tsl],
                            start=(dci == 0),
                            stop=(dci == DC - 1),
                        )
                        if first is None:
                            first = mm
                    if dob == 0 and tch == 0:
                        nc.scalar.activation(
                            gt[:, dob, tsl],
                            ps_g,
                            AF.Identity,
                            bias=w2_sb[:, hh, dob : dob + 1],
                        )
                    else:
                        nc.vector.tensor_scalar_add(
                            gt[:, dob, tsl], ps_g, w2_sb[:, hh, dob : dob + 1]
                        )
            return gt, first

        mt_cur = wpool.tile([128, DC, d], BF16, name="mt0")
        nc.sync.dma_start(out=mt_cur, in_=mt_d[0])
        nc.sync.dma_start(out=qt_sb[:, 1, :], in_=qt_d[:, 1, :])
        nc.sync.dma_start(out=qt8_sb, in_=qt8_d)
        gt_cur, first_mm = make_gt(0, mt_cur)

        for hh in range(h):
            # ---- per-head bulk loads (prefetched via wpool) -------------
            gated = []
            wvo_sb = wpool.tile([128, DC, d], BF16, name=f"wvo{hh}")
            gated.append(nc.sync.dma_start(out=wvo_sb, in_=wvot_d[hh]))
            f_bc = wpool.tile([128, t], BF16, name=f"f{hh}")
            nc.gpsimd.dma_start(out=f_bc, in_=_bcast(f_d[hh][None, :], 128))
            if hh == 0:
                for g in gated:
                    add_dep_helper(
                        g.ins, first_mm.ins, reason="defer bulk load past cold start"
                    )
            if hh + 1 < h:
                mt_next = wpool.tile([128, DC, d], BF16, name=f"mt{hh + 1}")
                nc.sync.dma_start(out=mt_next, in_=mt_d[hh + 1])

            # ---- scores + exp + f/rowsum + fused U + attn(db0) chase ----
            Ex = hpool.tile([128, SB, t], BF16)
            Uv = hpool.tile([128, SB, d], BF16)
            lsum = spool.tile([128, SB], F32)
            rr = spool.tile([128, SB], F32)
            pss0 = [
                ao_pool.tile([128, 512], F32, tag="ao", name=f"ao0_{i}")
                for i in range(TC)
            ]

            def attn_mms(pss, sc, dsl, stop):
                for tch in range(TC):
                    tsl = slice(tch * 512, (tch + 1) * 512)
                    nc.tensor.matmul(
                        pss[tch],
                        Uv[:, sc, dsl],
                        Ex[:, sc, tsl],
                        start=(sc == 0),
                        stop=stop,
                    )

            for sb in range(SB):
                ssl = slice(sb * 128, (sb + 1) * 128)
                at = at_pool.tile([128, TC * 512], F32, tag="at", name="at")
                # fp8 DoubleRow: contraction 256 in one pass, 256-wide out
                # chunks; start=True only on each bank's first MM (start
                # clears the whole 2KB bank region).
                for q4 in range(TC * 2):
                    nc.tensor.matmul(
                        at[:, q4 * 256 : (q4 + 1) * 256],
                        qt8_sb[:, 0:DC, ssl],
                        gt_cur[:, 0:DC, q4 * 256 : (q4 + 1) * 256],
                        start=(q4 % 2 == 0),
                        stop=(q4 % 2 == 1),
                        perf_mode=DR,
                        skip_group_check=True,
                    )
                # fused U projection for this s-block (fills PE bubbles);
                # bvo enters as a K=1 matmul row
                ps_u = mm_pool.tile([128, 512], F32, tag="mm")
                nc.tensor.matmul(
                    ps_u[:, :d],
                    ones_sb,
                    bvo_sb[:, hh, :],
                    start=True,
                    stop=False,
                    skip_group_check=True,
                )
                for dc in range(DC):
                    nc.tensor.matmul(
                        ps_u[:, :d],
                        qt_sb[:, dc, ssl],
                        wvo_sb[:, dc, :],
                        start=False,
                        stop=(dc == DC - 1),
                        skip_group_check=True,
                    )
                # attention (first d-block) chases the pipeline, lag 2
                if sb >= 2:
                    attn_mms(pss0, sb - 2, slice(0, 128), stop=False)
                g_sb = spool.tile([128, t], BF16, tag="g", name="g_sb", bufs=4)
                nc.scalar.activation(
                    g_sb,
                    at,
                    AF.Exp,
                    scale=scale,
                    bias=bc0_sb[:, hh : hh + 1],
                )
                # Ex = g*f and l = row-sum in ONE DVE pass
                nc.vector.scalar_tensor_tensor(
                    out=Ex[:, sb, :],
                    in0=g_sb,
                    scalar=1.0,
                    in1=f_bc,
                    op0=mybir.AluOpType.mult,
                    op1=mybir.AluOpType.mult,
                    accum_out=lsum[:, sb : sb + 1],
                )
                nc.vector.reciprocal(rr[:, sb : sb + 1], lsum[:, sb : sb + 1])
                if sb % 2 == 0:
                    nc.scalar.activation(
                        Uv[:, sb, :],
                        ps_u[:, :d],
                        AF.Copy,
                        scale=rr[:, sb : sb + 1],
                    )
                else:
                    nc.vector.tensor_scalar_mul(
                        Uv[:, sb, :], ps_u[:, :d], rr[:, sb : sb + 1]
                    )

            # ---- next head's G~ while ScalarE drains the exp tail -------
            if hh + 1 < h:
                gt_cur, _ = make_gt(hh + 1, mt_next)
                mt_cur = mt_next

            # ---- attn db0 tail, drains, then db1 ------------------------
            attn_mms(pss0, SB - 2, slice(0, 128), stop=False)
            attn_mms(pss0, SB - 1, slice(0, 128), stop=True)
            pss1 = [
                ao_pool.tile([128, 512], F32, tag="ao", name=f"ao1_{i}")
                for i in range(TC)
            ]
            for sc in range(SB):
                attn_mms(pss1, sc, slice(128, 256), stop=(sc == SB - 1))
            for db, pss in ((0, pss0), (1, pss1)):
                for tch in range(TC):
                    tsl = slice(tch * 512, (tch + 1) * 512)
                    if hh == 0:
                        nc.vector.tensor_copy(out_sb[:, db, tsl], pss[tch])
                    else:
                        nc.vector.tensor_add(
                            out_sb[:, db, tsl], out_sb[:, db, tsl], pss[tch]
                        )
                    if hh == h - 1:
                        nc.sync.dma_start(
                            out=out_d[:, db, tsl], in_=out_sb[:, db, tsl]
                        )

    nc.compile()
    return nc


_NC_CACHE = {}


def _get_nc(shape_key):
    if shape_key not in _NC_CACHE:
        _NC_CACHE[shape_key] = build_nc(*shape_key)
    return _NC_CACHE[shape_key]


def _pmajor(a, last):
    """[..., C*128, last] -> [..., 128, C, last] partition-major layout."""
    lead = a.shape[:-2]
    c = a.shape[-2] // 128
    return np.ascontiguousarray(
        a.reshape(*lead, c, 128, last).swapaxes(-3, -2)
    )


def _prep_inputs(Q, Wq, bq, Wk, bk, Wv, bv, Wo, bo):
    t, b, d = Q.shape
    h, e, _ = Wq.shape
    s = np.float32(1.0 / np.sqrt(e))
    bf = ml_dtypes.bfloat16
    Q = np.asarray(Q, np.float32)
    Wq = np.asarray(Wq, np.float32)
    Wk = np.asarray(Wk, np.float32)
    Wv = np.asarray(Wv, np.float32)
    Wo = np.asarray(Wo, np.float32)
    bq = np.asarray(bq, np.float32)
    bk = np.asarray(bk, np.float32)
    bv = np.asarray(bv, np.float32)

    # quadratic-form fold: scores need M, w2 (into G~), c1 -> f, c0 -> bias
    M = np.stack([Wq[i].T @ Wk[i] for i in range(h)])           # [H, D, D]
    w1 = np.stack([Wq[i].T @ bk[i] for i in range(h)])          # [H, D]
    w2 = np.stack([Wk[i].T @ bq[i] for i in range(h)])          # [H, D]
    c0 = np.array([bq[i] @ bk[i] for i in range(h)], np.float32)
    # fused V/O projection
    Wvo = np.stack([(Wo[:, i * e : (i + 1) * e] @ Wv[i]).T for i in range(h)])
    bvo = np.stack([Wo[:, i * e : (i + 1) * e] @ bv[i] for i in range(h)])

    qt_all = _pmajor(Q.transpose(1, 2, 0).astype(bf), t)        # [B,128,DC,T]
    f8 = ml_dtypes.float8_e4m3
    qt8_all = _pmajor(
        np.clip(Q.transpose(1, 2, 0), -240, 240).astype(f8), t
    )
    # f[t] = exp(s*c1[t]) per batch & head: c1 = Q @ w1
    f_all = np.exp(s * np.einsum("tbd,hd->bht", Q, w1)).astype(bf)  # [B,H,T]
    shared = {
        "mt": _pmajor(M.astype(bf), d),
        "wvot": _pmajor(Wvo.astype(bf), d),
        "w2s": np.ascontiguousarray(w2.reshape(h, -1, 128).transpose(2, 0, 1)),
        "bc0": np.ascontiguousarray(
            np.tile((s * c0)[None, :], (128, 1)).astype(np.float32)
        ),
        "bvo": np.ascontiguousarray(bvo.astype(bf)[None]),
    }
    in_maps = [
        {
            "qt": np.ascontiguousarray(qt_all[bb]),
            "qt8": np.ascontiguousarray(qt8_all[bb]),
            "fq": np.ascontiguousarray(f_all[bb]),
            **shared,
        }
        for bb in range(b)
    ]
    return in_maps, (t, d, h, e)


def kernel(Q, Wq, bq, Wk, bk, Wv, bv, Wo, bo, _trace=False):
    in_maps, (t, d, h, e) = _prep_inputs(Q, Wq, bq, Wk, bk, Wv, bv, Wo, bo)
    bo_f = np.asarray(bo, np.float32)
    nc = _get_nc((t, d, h, e))
    res = bass_utils.run_bass_kernel_spmd(
        nc, in_maps, core_ids=list(range(len(in_maps))), trace=_trace
    )
    # device output is OUT[d, t] partition-major: [128, DB, t]
    outs = []
    for b in range(len(in_maps)):
        arr = res.results[b]["out"]  # [128, DB, t]
        outs.append(arr.transpose(2, 1, 0).reshape(t, d) + bo_f)
    out = np.stack(outs, axis=1)
    if _trace:
        kernel.last_results = res
    return np.ascontiguousarray(out.astype(np.float32))
